# revision 20
# baseline (speedup 1.0000x reference)
"""HSTU multi-head attention kernel for 8 Trainium2 NeuronCores (v2).

Sharding: batch (4) x head-group (2 groups of 4 heads) -> 8 cores.
Per core: LN(x[b]) -> qk/uv projections (its 4 heads) -> silu ->
silu-attention with host-derived block schedule (skip / plain / masked)
-> batched per-head LN (stats on DVE, sqrt on ACT, recip_approx on DVE)
-> U-gate -> partial output projection.  Host sums the two head-group
partials per batch and adds x + o_b.

v2 structural changes vs v1 (trace-driven):
 - A/B/C stages interleaved per 512-token block so the PE stays dense
   (p-state ramp) and stage C starts early.
 - QK scores for a head-pair go into one 2-bank PSUM tile; ONE silu
   activation covers both heads (fewer ACT instructions).
 - Scores in bf16; V,U,normed-x in bf16 (faster DVE, same PE rate).
 - AV matmul in [query, head*d] orientation (st as stationary) so LN
   stats are free-dim reductions on DVE - no stat matmuls on PE.
 - All Sqrt work batched (stage A per-block + one stage-C2 pass) -> 3
   activation-table loads total instead of 31 (1283ns each).
 - DVE reciprocal (3157ns/instr measured) replaced by ACT Sqrt +
   reciprocal_approx_fast (custom DVE op).
 - Transposes via the DMA XBAR (dma_start_transpose), not the PE.

Algebraic folds (exact):
 - ln_w/ln_b folded into projection weights + bias.
 - scores/S scaling folded into LN eps: LN(v/S, eps) == LN(v, eps*S^2).
"""
import sys

sys.path.insert(0, "/opt/trn_rl_repo")

import numpy as np

HIDDEN = 512
NH = 8
DL = 64
DA = 64
EPS = 1e-6
B = 4
S = 2048
QCH = 512       # query chunk (free dim of scoresT)
KB = 128        # key block
NQC = S // QCH  # 4
NKB = S // KB   # 16
EPS_EFF = EPS * float(S) * float(S)  # fold 1/S into LN eps

_prog_cache = {}
_last_nc = None
_last_in_maps = None


def _bf16(a):
    import ml_dtypes
    return np.ascontiguousarray(a.astype(ml_dtypes.bfloat16))


def _build_schedule(attn_mask):
    """Classify each (chunk c, key block j) from the union over batches.

    Returns (sched, mask_tiles_per_batch):
      sched: tuple over c of tuple of (j, kind, uid); kind 0=plain, 1=masked
      mask_tiles_per_batch: list over b of np [n_uid, 128, 1024] bf16 tiles
        in scoresT orientation (k_local, q_local), duplicated along the
        free dim for the two heads sharing a paired-psum silu."""
    am = np.asarray(attn_mask)
    uid_map = {}
    per_batch_tiles = [[] for _ in range(B)]
    sched = []
    for c in range(NQC):
        row = []
        for j in range(NKB):
            blk = am[:, c * QCH:(c + 1) * QCH, j * KB:(j + 1) * KB]  # [B,512q,128k]
            if not blk.any():
                continue
            if blk.all():
                row.append((j, 0, -1))
                continue
            key = blk.tobytes()
            if key not in uid_map:
                uid_map[key] = len(uid_map)
                for b in range(B):
                    t = np.ascontiguousarray(blk[b].T.astype(np.float32))  # [128,512]
                    per_batch_tiles[b].append(np.concatenate([t, t], axis=1))
            row.append((j, 1, uid_map[key]))
        sched.append(tuple(row))
    sched = tuple(sched)
    masks = []
    for b in range(B):
        if per_batch_tiles[b]:
            masks.append(_bf16(np.stack(per_batch_tiles[b])))
        else:
            masks.append(_bf16(np.zeros((1, KB, 2 * QCH), np.float32)))
    return sched, masks


def _build_program(sched, n_uid, has_buv, lowering=True, parts=('a','b','c','c2b','c3','d')):
    import concourse.bass as bass
    import concourse.bacc as bacc
    import concourse.mybir as mybir
    from concourse.tile import TileContext
    from concourse.alu_op_type import AluOpType
    from contextlib import ExitStack
    import bass_rust

    f32 = mybir.dt.float32
    f32r = mybir.dt.float32r
    bf16 = mybir.dt.bfloat16
    AX = bass_rust.AxisListType.X
    ACT = mybir.ActivationFunctionType

    nc = bacc.Bacc("TRN2")

    xb = nc.declare_dram_parameter("xb", [S, HIDDEN], f32, isOutput=False)
    wqk = nc.declare_dram_parameter("wqk", [HIDDEN, 512], bf16, isOutput=False)
    wuv = nc.declare_dram_parameter("wuv", [HIDDEN, 512], bf16, isOutput=False)
    ow = nc.declare_dram_parameter("ow", [256, HIDDEN], bf16, isOutput=False)
    bqk = nc.declare_dram_parameter("bqk", [128, 4], f32, isOutput=False)
    if has_buv:
        buv = nc.declare_dram_parameter("buv", [128, 512], f32, isOutput=False)
    maskt = nc.declare_dram_parameter("maskt", [n_uid, KB, 2 * QCH], bf16,
                                      isOutput=False)
    yp = nc.declare_dram_parameter("yp", [S, HIDDEN], f32, isOutput=True)

    with nc.allow_low_precision(reason="bf16 activations are intended"), \
         TileContext(nc) as tc, ExitStack() as ctx:
        consts = ctx.enter_context(tc.tile_pool(name="consts", bufs=1))
        persist = ctx.enter_context(tc.tile_pool(name="persist", bufs=1))

        # ---- input tiles (DMAs emitted below, x[0:4] first) ----
        wqk_sb = [consts.tile([128, 512], bf16, tag=f"wqk{hc}",
                              name=f"wqk{hc}") for hc in range(4)]
        wuv_sb = [consts.tile([128, 512], bf16, tag=f"wuv{hc}",
                              name=f"wuv{hc}") for hc in range(4)]
        ow_sb = [consts.tile([128, HIDDEN], bf16, tag=f"ow{lc}",
                             name=f"ow{lc}") for lc in range(2)]
        bqk_sb = consts.tile([128, 4], f32, tag="bqk")
        if has_buv:
            buv_sb = consts.tile([128, 512], f32, tag="buv")
        mk_sb = [consts.tile([KB, 2 * QCH], bf16, tag=f"mk{u}",
                             name=f"mk{u}") for u in range(n_uid)]

        # ---- persistent activations ----
        # nxT[:, hc, s] = normed(x)[s, hc*128 + p] (bf16, transposed)
        nxT = persist.tile([128, 4, S], bf16, tag="nxT")
        # hT[t]: t in {q-hp0, q-hp1, k-hp0, k-hp1}; rows = 2 heads x 64d
        hT = [persist.tile([128, S], f32r, tag=f"hT{i}", name=f"hT{i}")
              for i in range(4)]
        # uvN[sb]: [128 seq, 0:256 = U(4 heads x 64), 256:512 = V]
        uvN = [persist.tile([128, 512], bf16, tag=f"uvN{i}", name=f"uvN{i}")
               for i in range(NKB)]
        # attention raw out (pre-LN), per qsub-pair: [128 q, 2 qsub x 4h x 64]
        outN = [persist.tile([128, 512], bf16, tag=f"outN{i}", name=f"outN{i}")
                for i in range(8)]
        gatedN = [persist.tile([128, 512], bf16, tag=f"g{i}", name=f"g{i}")
                  for i in range(8)]
        # u_dT[:, lc, s]: transposed gated output for the final projection
        u_dT = persist.tile([128, 2, S], bf16, tag="udT")
        # LN stats: 16 qsubs x 4 heads = 64 cols
        s1all = persist.tile([128, 64], f32, tag="s1all")
        s2all = persist.tile([128, 64], f32, tag="s2all")
        rsall = persist.tile([128, 64], f32, tag="rsall")
        mursall = persist.tile([128, 64], f32, tag="mursall")

        xts = [persist.tile([128, HIDDEN], f32, tag=f"xt{i}", name=f"xt{i}")
               for i in range(16)]

        pa = ctx.enter_context(tc.tile_pool(name="stA", bufs=2))
        pnorm = ctx.enter_context(tc.tile_pool(name="pnorm", bufs=3))
        pst = ctx.enter_context(tc.tile_pool(name="pst", bufs=4))
        psB = ctx.enter_context(tc.tile_pool(name="psB", bufs=2, space="PSUM"))
        psS = ctx.enter_context(tc.tile_pool(name="psS", bufs=2, space="PSUM"))
        psAcc = ctx.enter_context(tc.tile_pool(name="psAcc", bufs=1,
                                               space="PSUM"))

        astat = persist.tile([128, 16], f32, tag="astat")      # sum(x)
        astat2 = persist.tile([128, 16], f32, tag="astat2")    # sum(x^2)
        negmu = persist.tile([128, 16], f32, tag="negmu")
        rstd = persist.tile([128, 16], f32, tag="rstd")

        eps_a = persist.tile([128, 1], f32, tag="eps_a")
        nc.vector.memset(eps_a, EPS)
        eps_c = persist.tile([128, 1], f32, tag="eps_c")
        nc.vector.memset(eps_c, EPS_EFF)

        # ---- DMA emission order: first x blk0, then weights, then rest ----
        for sb in range(4):
            nc.sync.dma_start(out=xts[sb], in_=xb[sb * 128:(sb + 1) * 128, :])
        for hc in range(4):
            nc.sync.dma_start(out=wqk_sb[hc], in_=wqk[hc * 128:(hc + 1) * 128, :])
            nc.sync.dma_start(out=wuv_sb[hc], in_=wuv[hc * 128:(hc + 1) * 128, :])
        for lc in range(2):
            nc.sync.dma_start(out=ow_sb[lc], in_=ow[lc * 128:(lc + 1) * 128, :])
        nc.sync.dma_start(out=bqk_sb, in_=bqk[:, :])
        if has_buv:
            nc.sync.dma_start(out=buv_sb, in_=buv[:, :])
        for u in range(n_uid):
            nc.sync.dma_start(out=mk_sb[u], in_=maskt[u, :, :])
        for sb in range(4, 16):
            nc.sync.dma_start(out=xts[sb], in_=xb[sb * 128:(sb + 1) * 128, :])

        def a_stats(sbs):
            """Per-token LN stats: s1 on DVE, s2 via ACT Square-with-accum
            ('square' lives in every act table - no table load)."""
            for sb in sbs:
                nc.vector.reduce_sum(astat[:, sb:sb + 1], xts[sb], axis=AX)
                sq = pa.tile([128, HIDDEN], f32, tag="sq")
                nc.scalar.activation(sq, xts[sb], ACT.Square,
                                     accum_out=astat2[:, sb:sb + 1])

        def a_factors(sl):
            """negmu/rstd for a slice of seq-blocks (one ACT Sqrt)."""
            n = sl.stop - sl.start
            nc.vector.tensor_single_scalar(negmu[:, sl], astat[:, sl],
                                           -1.0 / HIDDEN, AluOpType.mult)
            m2 = pa.tile([128, n], f32, tag="m2")
            nc.vector.tensor_mul(m2, negmu[:, sl], negmu[:, sl])
            var = pa.tile([128, n], f32, tag="var")
            nc.vector.scalar_tensor_tensor(var, astat2[:, sl], 1.0 / HIDDEN,
                                           m2, AluOpType.mult,
                                           AluOpType.subtract)
            std = pa.tile([128, n], f32, tag="std")
            nc.scalar.activation(std, var, ACT.Sqrt, bias=eps_a)
            nc.vector.reciprocal_approx_fast(out=rstd[:, sl], in_=std)

        def a_norm(sbs):
            for sb in sbs:
                normed = pnorm.tile([128, HIDDEN], bf16, tag="normed")
                nc.vector.tensor_scalar(normed, xts[sb],
                                        negmu[:, sb:sb + 1],
                                        rstd[:, sb:sb + 1],
                                        AluOpType.add, AluOpType.mult)
                # XBAR transpose: out[p, hc, s'] = normed[s', hc*128+p]
                nc.sync.dma_start_transpose(
                    out=nxT[:, :, sb * 128:(sb + 1) * 128], in_=normed)

        def stage_a(blk, sub=3):
            a_stats(range(blk * 4, blk * 4 + 4))
            a_factors(slice(blk * 4, blk * 4 + 4))
            a_norm(range(blk * 4, blk * 4 + 4))

        def stage_b(blk):
            sc = blk
            # q/k projections for this seq chunk: 4 output tiles [128hd, 512s]
            for ob in range(4):
                pp = psB.tile([128, QCH], f32, tag="pp")
                for hc in range(4):
                    nc.tensor.matmul(
                        pp,
                        lhsT=wqk_sb[hc][:, ob * 128:(ob + 1) * 128],
                        rhs=nxT[:, hc, sc * QCH:(sc + 1) * QCH],
                        start=(hc == 0), stop=(hc == 3))
                nc.scalar.activation(hT[ob][:, sc * QCH:(sc + 1) * QCH],
                                     pp, ACT.Silu, bias=bqk_sb[:, ob:ob + 1])
            # u/v projections for the 4 seq blocks of this chunk
            for sb in range(blk * 4, blk * 4 + 4):
                pp = psB.tile([128, 512], f32, tag="pp")
                for hc in range(4):
                    nc.tensor.matmul(
                        pp,
                        lhsT=nxT[:, hc, sb * 128:(sb + 1) * 128],
                        rhs=wuv_sb[hc],
                        start=(hc == 0), stop=(hc == 3))
                if has_buv:
                    nc.vector.tensor_add(pp, pp, buv_sb)
                nc.scalar.activation(uvN[sb], pp, ACT.Silu)

        def emit_qk(c, j, kind, uid):
            """QK matmuls + paired silu (+ mask) for one key block.
            Returns the two st tiles (hp0, hp1), each [128k, 1024] bf16."""
            sts = []
            for hp in range(2):
                ps = psS.tile([KB, 2 * QCH], f32, tag="pspair")
                for hh in range(2):
                    p0 = 64 * hh
                    nc.tensor.matmul(
                        ps[:, hh * QCH:(hh + 1) * QCH],
                        lhsT=hT[2 + hp][p0:p0 + 64, j * KB:(j + 1) * KB],
                        rhs=hT[hp][p0:p0 + 64, c * QCH:(c + 1) * QCH],
                        start=True, stop=True)
                st = pst.tile([KB, 2 * QCH], bf16, tag="st")
                nc.scalar.activation(st, ps, ACT.Silu)
                if kind == 1:
                    nc.vector.tensor_mul(st, st, mk_sb[uid])
                sts.append(st)
            return sts

        av_started = {}

        def emit_av(c, j, sts, first, last, accP):
            for qsub in range(4):
                acc = accP[qsub // 2]
                half = qsub % 2
                for h in range(4):
                    hp, hh = h // 2, h % 2
                    key = (c, qsub // 2)
                    start = not av_started.get(key, False)
                    av_started[key] = True
                    nc.tensor.matmul(
                        acc[:, half * 256 + h * 64: half * 256 + (h + 1) * 64],
                        lhsT=sts[hp][:, hh * QCH + qsub * 128:
                                     hh * QCH + qsub * 128 + 128],
                        rhs=uvN[j][:, 256 + h * 64: 256 + (h + 1) * 64],
                        start=start, stop=last,
                        skip_group_check=True)

        def stage_c(c):
            js = sched[c]
            accP = [psAcc.tile([128, 512], f32, tag=f"accP{p}",
                               name=f"accP_{c}_{p}") for p in range(2)]
            if not js:
                for p in range(2):
                    nc.vector.memset(outN[c * 2 + p], 0.0)
                    nc.vector.memset(s1all[:, (c * 2 + p) * 8:
                                           (c * 2 + p) * 8 + 8], 0.0)
                    nc.vector.memset(s2all[:, (c * 2 + p) * 8:
                                           (c * 2 + p) * 8 + 8], 0.0)
                return
            pending = None
            for idx, (j, kind, uid) in enumerate(js):
                sts = emit_qk(c, j, kind, uid)
                if pending is not None:
                    emit_av(c, *pending, accP)
                pending = (j, sts, idx == 0, idx == len(js) - 1)
            emit_av(c, *pending, accP)
            # C2: copy-out then stats from SBUF (3D-AP reduce from PSUM is
            # rejected by walrus; SBUF is fine), per qsub-pair
            for p in range(2):
                t = c * 2 + p
                nc.vector.tensor_copy(outN[t], accP[p])
                nc.vector.reduce_sum(
                    s1all[:, t * 8:t * 8 + 8],
                    outN[t][:, :].rearrange("p (g d) -> p g d", d=64), axis=AX)
                sq = pa.tile([128, 512], bf16, tag="csq")
                nc.vector.tensor_mul(sq, outN[t], outN[t])
                nc.vector.reduce_sum(
                    s2all[:, t * 8:t * 8 + 8],
                    sq[:, :].rearrange("p (g d) -> p g d", d=64), axis=AX)

        # ---------------- main interleaved pipeline ----------------
        # blk0 stats fast-path feeds B(0) early; remaining stats batch
        # during B(0)/C(0) (their Sqrt costs one extra table flip).
        a_stats(range(0, 4))
        a_factors(slice(0, 4))
        a_norm(range(0, 4))
        stage_b(0)
        stage_c(0)
        a_stats(range(4, 16))
        a_factors(slice(4, 16))
        for blk in range(1, NQC):
            a_norm(range(blk * 4, blk * 4 + 4))
            stage_b(blk)
            stage_c(blk)

        # ---- C2b: batched LN scale factors ----
        if 'c2b' in parts:
            mu = pa.tile([128, 64], f32, tag="mu", bufs=1)
            nc.vector.tensor_single_scalar(mu, s1all, 1.0 / DL, AluOpType.mult)
            m2c = pa.tile([128, 64], f32, tag="m2c", bufs=1)
            nc.vector.tensor_mul(m2c, mu, mu)
            varc = pa.tile([128, 64], f32, tag="varc", bufs=1)
            nc.vector.scalar_tensor_tensor(varc, s2all, 1.0 / DL, m2c,
                                           AluOpType.mult, AluOpType.subtract)
            stdc = pa.tile([128, 64], f32, tag="stdc", bufs=1)
            nc.scalar.activation(stdc, varc, ACT.Sqrt, bias=eps_c)
            nc.vector.reciprocal_approx_fast(out=rsall, in_=stdc)
            nc.vector.tensor_mul(mursall, mu, rsall)

        # ---- C3 + D interleaved: normalize/gate tile t, transpose its two
        # qsubs, then immediately project those two query blocks (yt copies
        # on ACT - idle in the tail, 'copy' needs no table load) ----
        def stage_d(qb):
            py = psB.tile([128, HIDDEN], f32, tag="pp")
            nc.tensor.matmul(py,
                             lhsT=u_dT[:, 0, qb * 128:(qb + 1) * 128],
                             rhs=ow_sb[0], start=True, stop=False)
            nc.tensor.matmul(py,
                             lhsT=u_dT[:, 1, qb * 128:(qb + 1) * 128],
                             rhs=ow_sb[1], start=False, stop=True)
            yt = pnorm.tile([128, HIDDEN], f32, tag="yt")
            nc.scalar.copy(yt, py)
            nc.sync.dma_start(out=yp[qb * 128:(qb + 1) * 128, :], in_=yt)

        for t in range(8):
            for half in range(2):
                qsub = t * 2 + half
                for h in range(4):
                    col = qsub * 4 + h
                    nc.vector.tensor_scalar(
                        gatedN[t][:, half * 256 + h * 64:
                                  half * 256 + (h + 1) * 64],
                        outN[t][:, half * 256 + h * 64:
                                half * 256 + (h + 1) * 64],
                        rsall[:, col:col + 1], mursall[:, col:col + 1],
                        AluOpType.mult, AluOpType.subtract)
            for half in range(2):
                qsub = t * 2 + half
                nc.vector.tensor_mul(
                    gatedN[t][:, half * 256:(half + 1) * 256],
                    gatedN[t][:, half * 256:(half + 1) * 256],
                    uvN[qsub][:, 0:256])
                nc.sync.dma_start_transpose(
                    out=u_dT[:, :, qsub * 128:(qsub + 1) * 128],
                    in_=gatedN[t][:, half * 256:(half + 1) * 256])
            stage_d(t * 2)
            stage_d(t * 2 + 1)

    if lowering:
        nc.compile()
    return nc


def _core_inputs(x, uvqk_eff, bias_full, o_w, masks):
    """Per-core input maps."""
    in_maps = []
    has_buv = not np.allclose(bias_full[:512], 0.0)
    for core in range(8):
        b, g = core // 2, core % 2
        heads = [4 * g + i for i in range(4)]
        # wqk cols: [q-hp0 (128) | q-hp1 | k-hp0 | k-hp1]
        qc0 = [1024 + 64 * h + d for h in heads[0:2] for d in range(64)]
        qc1 = [1024 + 64 * h + d for h in heads[2:4] for d in range(64)]
        kc0 = [1536 + 64 * h + d for h in heads[0:2] for d in range(64)]
        kc1 = [1536 + 64 * h + d for h in heads[2:4] for d in range(64)]
        qksel = qc0 + qc1 + kc0 + kc1
        uc = [0 + 64 * h + d for h in heads for d in range(64)]
        vc = [512 + 64 * h + d for h in heads for d in range(64)]
        uvsel = uc + vc
        wqk_c = np.ascontiguousarray(uvqk_eff[:, qksel])
        wuv_c = np.ascontiguousarray(uvqk_eff[:, uvsel])
        bqk_c = np.ascontiguousarray(bias_full[qksel].reshape(4, 128).T)
        lsel = [64 * h + d for h in heads for d in range(64)]
        owc = np.ascontiguousarray(o_w[lsel, :])
        m = {
            "xb": np.ascontiguousarray(x[b]),
            "wqk": _bf16(wqk_c), "wuv": _bf16(wuv_c), "ow": _bf16(owc), "bqk": bqk_c,
            "maskt": masks[b],
        }
        if has_buv:
            m["buv"] = np.ascontiguousarray(
                np.broadcast_to(bias_full[uvsel][None, :], (128, 512)).astype(
                    np.float32))
        in_maps.append(m)
    return in_maps


def kernel(x, attn_mask, uvqk, o_w, o_b, ln_w, ln_b):
    global _last_nc, _last_in_maps
    x = np.asarray(x, np.float32)
    uvqk = np.asarray(uvqk, np.float32)
    o_w = np.asarray(o_w, np.float32)
    o_b = np.asarray(o_b, np.float32)
    ln_w = np.asarray(ln_w, np.float32)
    ln_b = np.asarray(ln_b, np.float32)

    sched, masks = _build_schedule(attn_mask)
    uvqk_eff = ln_w[:, None] * uvqk
    bias_full = ln_b @ uvqk

    has_buv = not np.allclose(bias_full[:512], 0.0)
    n_uid = masks[0].shape[0]
    key = (sched, n_uid, has_buv)
    if key not in _prog_cache:
        _prog_cache[key] = _build_program(sched, n_uid, has_buv)
    nc = _prog_cache[key]

    in_maps = _core_inputs(x, uvqk_eff, bias_full, o_w, masks)
    _last_nc, _last_in_maps = nc, in_maps

    from concourse.bass_utils import run_bass_kernel_spmd
    res = run_bass_kernel_spmd(nc, in_maps, list(range(8)))
    outs = res.results

    y = np.empty((B, S, HIDDEN), np.float32)
    for b in range(B):
        y[b] = x[b] + o_b[None, :] + outs[2 * b]["yp"] + outs[2 * b + 1]["yp"]
    return y
